# revision 48
# baseline (speedup 1.0000x reference)
"""GCNConv on 8 Trainium2 NeuronCores. Self-contained graded kernel."""


import sys
from contextlib import ExitStack
from dataclasses import dataclass

import ml_dtypes
import numpy as np

sys.path.insert(0, "/opt/trn_rl_repo")

import concourse.bacc as bacc  # noqa: E402
import concourse.mybir as mybir  # noqa: E402

BF16 = ml_dtypes.bfloat16
FP8 = ml_dtypes.float8_e4m3


@dataclass(frozen=True)
class P:
    n_nodes: int = 100000
    d: int = 128
    n_cores: int = 8
    npc: int = 12500          # nodes per core
    bd: int = 500             # destinations per bank
    nb: int = 25              # banks per core
    win: int = 32             # max dests per window (psum column block)
    nwin: int = 16            # windows per bank; nwin*win = psum bank cols

    @property
    def cols(self):
        return self.nwin * self.win


FULL = P()


def _pack_bank(cnt, nwin, win, targets=None):
    """Assign len(cnt) dests into nwin bins (<=win dests each): worst-fit
    decreasing toward the given per-bin load targets, then repair toward a
    sorted profile of [512]*(nwin-1)+[384] so subcap sums to 4*(nwin-1)+3.
    Returns (bin id per dest, bin loads desc)."""
    nd = len(cnt)
    order = np.argsort(-cnt, kind="stable")
    if targets is None:
        targets = [512] * (nwin - 1) + [384]
    rem_e = np.asarray(targets, np.int64).copy()
    rem_d = np.full(nwin, win, np.int64)
    sums = np.zeros(nwin, np.int64)
    assign = np.empty(nd, np.int64)
    NEG = -1 << 40
    for i in order:
        c = int(cnt[i])
        feas = rem_d > 0
        b = int(np.argmax(np.where(feas, rem_e, NEG)))
        assign[i] = b
        sums[b] += c
        rem_e[b] -= c
        rem_d[b] -= 1
    counts = np.bincount(assign, minlength=nwin)
    # repair >512 bins: plain move if a bin has count room (respecting the
    # lightest bin's 384 budget), else swap a dest for a smaller one from a
    # bin with load slack (counts unchanged)
    for _ in range(64):
        wsrc = int(np.argmax(sums))
        over = int(sums[wsrc]) - 512
        if over <= 0:
            break
        mem = np.where(assign == wsrc)[0]
        cand = mem[np.argsort(cnt[mem], kind="stable")]
        dst_order = np.argsort(sums, kind="stable")
        moved = False
        for i in cand:                       # plain move
            c = int(cnt[i])
            if c < over:
                continue
            for wdst in dst_order:
                if wdst == wsrc or counts[wdst] >= win:
                    continue
                lim = 384 if wdst == dst_order[0] else 512
                if sums[wdst] + c <= lim:
                    assign[i] = wdst
                    sums[wsrc] -= c
                    sums[wdst] += c
                    counts[wsrc] -= 1
                    counts[wdst] += 1
                    moved = True
                    break
            if moved:
                break
        if not moved:                        # swap
            for wdst in dst_order:
                if wdst == wsrc:
                    continue
                lim = 384 if wdst == dst_order[0] else 512
                slack = lim - int(sums[wdst])
                if slack < over:
                    continue
                dmem = np.where(assign == wdst)[0]
                for i in cand:
                    ci = int(cnt[i])
                    ks = dmem[(cnt[dmem] <= ci - over)
                              & (cnt[dmem] >= ci - slack)]
                    if len(ks):
                        k = ks[np.argmax(cnt[ks])]   # minimal delta
                        ck = int(cnt[k])
                        assign[i], assign[k] = wdst, wsrc
                        sums[wsrc] += ck - ci
                        sums[wdst] += ci - ck
                        moved = True
                        break
                if moved:
                    break
        if not moved:
            break
    binorder = np.argsort(-sums, kind="stable")
    remap = np.empty(nwin, np.int64)
    remap[binorder] = np.arange(nwin)
    return remap[assign], sums[binorder]


def host_prep(x, edge_index, W, b, p: P):
    """Build per-core device inputs. Returns (in_maps, colmap, subcap)."""
    n, d = p.n_nodes, p.d
    row = np.asarray(edge_index[0]).astype(np.int64)
    col = np.asarray(edge_index[1]).astype(np.int64)
    x = np.asarray(x, np.float32)
    E = row.shape[0]
    ngb = p.n_cores * p.nb

    deg = np.bincount(row, minlength=n).astype(np.float32)
    dis = np.where(deg > 0, deg ** -0.5, 0.0).astype(np.float32)
    norm = (dis[row] * dis[col]).astype(np.float32)

    # LPT-balance dests across each core's banks so every bank carries
    # ~E/(cores*nb) edges: subcap is a global max, so variance = padding.
    import heapq
    degi = np.bincount(row, minlength=n)
    bank_of = np.empty(n, np.int64)         # global bank id per node
    pos_of = np.empty(n, np.int64)          # position within bank
    for c in range(p.n_cores):
        lo = c * p.npc
        order_c = np.argsort(-degi[lo:lo + p.npc], kind="stable")
        heap = [(0, bi) for bi in range(p.nb)]
        cnt = np.zeros(p.nb, np.int64)
        for v in order_c:
            while True:
                load, bi = heapq.heappop(heap)
                if cnt[bi] < p.bd:
                    break
            bank_of[lo + v] = c * p.nb + bi
            pos_of[lo + v] = cnt[bi]
            cnt[bi] += 1
            heapq.heappush(heap, (load + int(degi[lo + v]), bi))

    gb = bank_of[row]                       # global bank id
    dloc = pos_of[row]                      # dest within bank

    # pack each bank's dests into windows
    degb = np.bincount(gb * p.bd + dloc, minlength=ngb * p.bd).reshape(ngb, p.bd)
    wof = np.empty((ngb, p.bd), np.int64)   # window of dest
    jof = np.empty((ngb, p.bd), np.int64)   # col within window
    bank_bins = np.empty((ngb, p.nwin), np.int64)
    for g in range(ngb):
        wo, sums = _pack_bank(degb[g], p.nwin, p.win)
        wof[g] = wo
        bank_bins[g] = sums
        o = np.argsort(wo, kind="stable")
        starts = np.zeros(p.nwin, np.int64)
        cnts = np.bincount(wo, minlength=p.nwin)
        starts[1:] = np.cumsum(cnts)[:-1]
        r = np.empty(p.bd, np.int64)
        r[o] = np.arange(p.bd) - starts[wo[o]]
        jof[g] = r
    assert (jof < p.win).all()

    # data-derived per-window-index sub counts (shared across cores/banks)
    subcap = np.maximum(1, -(-bank_bins.max(axis=0) // 128)).astype(np.int64)
    spb = int(subcap.sum())
    subbase = np.zeros(p.nwin, np.int64)
    subbase[1:] = np.cumsum(subcap)[:-1]

    # per-edge window / slot
    ew = wof[gb, dloc]
    ej = jof[gb, dloc]
    cell = gb * p.nwin + ew
    order = np.argsort(cell, kind="stable")
    cell_s = cell[order]
    col_s = col[order]
    norm_s = norm[order]
    ej_s = ej[order]
    gb_s = gb[order]
    ew_s = ew[order]

    cell_counts = np.bincount(cell, minlength=ngb * p.nwin)
    assert (cell_counts.reshape(ngb, p.nwin) <= subcap[None, :] * 128).all()
    cell_starts = np.zeros(ngb * p.nwin, np.int64)
    cell_starts[1:] = np.cumsum(cell_counts)[:-1]
    rank = np.arange(E) - cell_starts[cell_s]
    slot = subbase[ew_s] * 128 + rank       # slot within bank

    # norm folded into the gathered rows (single fp8 quantization); S is 0/1.
    # G and S are fused into one per-bank fp8 blob: per partition, per sub:
    # [d gathered feats | win one-hot] so one DMA streams both.
    slots = spb * 128
    G_all = np.zeros((ngb, slots, d), FP8)
    G_all[gb_s, slot] = (x[col_s] * norm_s[:, None]).astype(FP8)
    G_all = G_all.reshape(ngb, spb, 128, d).transpose(0, 2, 1, 3)

    S = np.zeros((ngb, 128, spb, p.win), FP8)
    sub = subbase[ew_s] + rank // 128
    pslot = rank % 128
    S[gb_s, pslot, sub, ej_s] = 1.0
    blob = np.concatenate([G_all, S], axis=3)  # [ngb, 128, spb, d+win]
    blob = blob.reshape(ngb, 128, spb * (d + p.win))

    # column map: (gb, 32*w + j) -> dest local id within core, else -1
    node_of = np.empty((ngb, p.bd), np.int64)
    allv = np.arange(n)
    node_of[bank_of, pos_of] = allv % p.npc
    colmap = np.full((ngb, p.cols), -1, np.int64)
    gidx = np.repeat(np.arange(ngb), p.bd)
    colmap[gidx, (wof * p.win + jof).ravel()] = node_of.ravel()
    colmap = colmap.reshape(p.n_cores, p.nb, p.cols)

    Wt = np.ascontiguousarray(np.asarray(W, np.float32).T).astype(BF16)
    bias = np.asarray(b, np.float32).reshape(d, 1)

    in_maps = []
    for c in range(p.n_cores):
        xc = x[c * p.npc:(c + 1) * p.npc]           # [npc, d]
        cm = colmap[c].reshape(-1)                   # [nb*cols]
        xTp = np.zeros((p.nb * p.cols, d), np.float32)
        used = cm >= 0
        xTp[used] = xc[cm[used]]
        xTp = np.ascontiguousarray(
            xTp.reshape(p.nb, p.cols, d).transpose(0, 2, 1)).astype(BF16)
        # self-connection rows ride the fp8 blob as raw bytes (bitcast on dev)
        xT8 = xTp.view(np.uint8).view(FP8).reshape(p.nb, 128, 2 * p.cols)
        Bc = np.concatenate([blob[c * p.nb:(c + 1) * p.nb], xT8], axis=2)
        in_maps.append({
            "B": np.ascontiguousarray(
                Bc.transpose(1, 0, 2).reshape(128, -1)),
            "Wt": Wt,
            "bias": bias,
        })
    return in_maps, colmap, subcap


def assemble(results, p: P, colmap):
    out = np.empty((p.n_cores * p.npc, p.d), np.float32)
    for c in range(p.n_cores):
        o = np.asarray(results[c]["outT"])          # [npair, d, 2*cols]
        o = o.reshape(-1, p.d, 2, p.cols).transpose(0, 2, 1, 3)
        o = o.reshape(-1, p.d, p.cols)[:p.nb]       # [nb, d, cols]
        o = o.transpose(0, 2, 1).reshape(-1, p.d)   # [nb*cols, d]
        cm = colmap[c].reshape(-1)
        used = cm >= 0
        out[c * p.npc + cm[used]] = o[used]
    return out


def build_kernel(p: P, subcap):
    nc = bacc.Bacc("TRN2", debug=False)
    dt = mybir.dt
    nbk, win, d, cols = p.nb, p.win, p.d, p.cols
    subcap = [int(v) for v in subcap]
    spb = sum(subcap)
    subbase = [0] * p.nwin
    for w in range(1, p.nwin):
        subbase[w] = subbase[w - 1] + subcap[w - 1]
    window_of_sub = []
    for w in range(p.nwin):
        window_of_sub += [w] * subcap[w]

    dw = d + win
    bpb = spb * dw + 2 * cols        # blob elems per partition per bank
    # variable chunk schedule: small chunks at start (fast ramp) + end (short
    # tail), 2-bank chunks in the middle; 5 buffers deep
    chunks = [1, 1] + [2] * ((nbk - 5) // 2) + [1, 1, 1]
    assert sum(chunks) == nbk
    nch = len(chunks)
    cstart = [0]
    for csz in chunks:
        cstart.append(cstart[-1] + csz)
    chunk_of = []
    for ci, csz in enumerate(chunks):
        chunk_of += [ci] * csz
    CMAX = max(chunks)

    B_d = nc.dram_tensor("B", [128, nbk * bpb], dt.float8e4, kind="ExternalInput")
    Wt_d = nc.dram_tensor("Wt", [d, d], dt.bfloat16, kind="ExternalInput")
    b_d = nc.dram_tensor("bias", [d, 1], dt.float32, kind="ExternalInput")
    npair = (nbk + 1) // 2
    out_d = nc.dram_tensor("outT", [npair, d, 2 * cols], dt.bfloat16,
                           kind="ExternalOutput")

    with ExitStack() as ctx:
        def sb(name, shape, dtype):
            return ctx.enter_context(nc.sbuf_tensor(name, shape, dtype))

        NB = 7                       # input-side buffer depth (chunks)
        Bsb = [sb(f"Bsb{i}", [128, CMAX * bpb], dt.float8e4) for i in range(NB)]
        ax = [sb(f"ax{i}", [128, cols], dt.bfloat16) for i in range(3)]
        # output staging: 2 ping-pong buffers x 2 banks each
        osb = [sb(f"osb{i}", [128, 2 * cols], dt.bfloat16) for i in range(2)]
        Wt_sb = sb("Wt_sb", [128, d], dt.bfloat16)
        b_sb = sb("b_sb", [128, 1], dt.float32)
        pagg = [ctx.enter_context(nc.psum_tensor(f"pagg{i}", [128, cols], dt.float32))
                for i in range(3)]
        pfin = [ctx.enter_context(nc.psum_tensor(f"pfin{i}", [128, cols], dt.float32))
                for i in range(3)]

        names = ["s_const", "s_peb", "s_dve", "s_fin", "s_act"]
        sem = {nm: ctx.enter_context(nc.semaphore(nm)) for nm in names}
        # parity-split DMA sems: one sem per buffer slot, so at most one
        # in-flight DMA per sem and wait values are unambiguous
        sem["s_b"] = [ctx.enter_context(nc.semaphore(f"s_b{i}")) for i in range(NB)]
        sem["s_b0a"] = ctx.enter_context(nc.semaphore("s_b0a"))
        sem["s_bza"] = ctx.enter_context(nc.semaphore("s_bza"))
        sem["s_out"] = [ctx.enter_context(nc.semaphore(f"s_out{i}")) for i in range(2)]
        half0 = (spb // 2) * dw      # bank-0 first-half split point (elems)

        with nc.Block() as block:
            @block.sync
            def _(s):
                for j in range(nch):
                    if j >= NB:
                        # buffer j%NB free once DVE consumed chunk j-NB fully
                        s.wait_ge(sem["s_dve"], cstart[j - NB] + chunks[j - NB])
                    lo, hi = cstart[j], cstart[j + 1]
                    if j == 0:
                        # split bank 0 so the PE can start on the first half
                        s.dma_start(Bsb[0][:, 0:half0], B_d[:, 0:half0]
                                    ).then_inc(sem["s_b0a"], 16)
                        s.dma_start(Bsb[0][:, half0:bpb], B_d[:, half0:bpb]
                                    ).then_inc(sem["s_b"][0], 16)
                        continue
                    if j == nch - 1:
                        # split the last bank too: PE starts on its first half
                        # while the second half streams
                        zb = lo * bpb
                        s.dma_start(Bsb[j % NB][:, 0:half0],
                                    B_d[:, zb:zb + half0]
                                    ).then_inc(sem["s_bza"], 16)
                        s.dma_start(Bsb[j % NB][:, half0:bpb],
                                    B_d[:, zb + half0:zb + bpb]
                                    ).then_inc(sem["s_b"][j % NB], 16)
                        continue
                    s.dma_start(Bsb[j % NB][:, 0:(hi - lo) * bpb],
                                B_d[:, lo * bpb:hi * bpb]
                                ).then_inc(sem["s_b"][j % NB], 16)

            @block.tensor
            def _(pe):
                def final_mm(fb):
                    pe.wait_ge(sem["s_dve"], fb + 1)
                    if fb == 0:
                        pe.wait_ge(sem["s_const"], 32)
                    if fb >= 3:
                        pe.wait_ge(sem["s_act"], fb - 2)
                    nc.tensor.matmul(
                        pfin[fb % 3][:, :], Wt_sb[:, :], ax[fb % 3][:, :],
                        start=True, stop=True,
                    ).then_inc(sem["s_fin"], 1)

                for bk in range(nbk):
                    cj = chunk_of[bk]
                    lane = bk - cstart[cj]
                    if bk == 0:
                        pe.wait_ge(sem["s_b0a"], 16)
                    elif bk == nbk - 1:
                        pe.wait_ge(sem["s_bza"], 16)
                    elif lane == 0:
                        pe.wait_ge(sem["s_b"][cj % NB], 16 * (cj // NB + 1))
                    if bk >= 3:
                        pe.wait_ge(sem["s_dve"], bk - 2)
                    base = lane * bpb
                    mm = None
                    for si in range(spb):
                        if (bk == 0 or bk == nbk - 1) and si == spb // 2:
                            pe.wait_ge(sem["s_b"][(0 if bk == 0 else
                                                   (nch - 1) % NB)],
                                       16 * (1 if bk == 0 else
                                             ((nch - 1) // NB + 1)))
                        w = window_of_sub[si]
                        jj = si - subbase[w]
                        mm = nc.tensor.matmul(
                            pagg[bk % 3][:, w * win:(w + 1) * win],
                            Bsb[cj % NB][:, base + si * dw:base + si * dw + d],
                            Bsb[cj % NB][:, base + si * dw + d:base + (si + 1) * dw],
                            start=(jj == 0), stop=(jj == subcap[w] - 1),
                        )
                    mm.then_inc(sem["s_peb"], 1)
                    if bk >= 1:
                        final_mm(bk - 1)
                final_mm(nbk - 1)

            @block.vector
            def _(v):
                for bk in range(nbk):
                    cj = chunk_of[bk]
                    base = (bk - cstart[cj]) * bpb + spb * dw
                    v.wait_ge(sem["s_peb"], bk + 1)
                    if bk >= 3:
                        v.wait_ge(sem["s_fin"], bk - 2)
                    xv = Bsb[cj % NB][:, base:base + 2 * cols].bitcast(dt.bfloat16)
                    nc.vector.tensor_add(
                        ax[bk % 3][:, :], pagg[bk % 3][:, :], xv
                    ).then_inc(sem["s_dve"], 1)

            @block.scalar
            def _(a):
                a.dma_start(Wt_sb[:, :], Wt_d[:, :]).then_inc(sem["s_const"], 16)
                a.dma_start(b_sb[:, :], b_d[:, :]).then_inc(sem["s_const"], 16)
                a.wait_ge(sem["s_const"], 32)
                for bk in range(nbk):
                    pi, lane = bk // 2, bk % 2
                    a.wait_ge(sem["s_fin"], bk + 1)
                    if lane == 0 and pi >= 2:
                        a.wait_ge(sem["s_out"][pi % 2], 16 * (pi // 2))
                    nc.scalar.activation(
                        osb[pi % 2][:, lane * cols:(lane + 1) * cols],
                        pfin[bk % 3][:, :],
                        mybir.ActivationFunctionType.Identity, bias=b_sb[:, :],
                    ).then_inc(sem["s_act"], 1)
                    if lane == 1:
                        a.wait_ge(sem["s_act"], bk + 1)
                        a.dma_start(out_d[pi], osb[pi % 2][:, :]
                                    ).then_inc(sem["s_out"][pi % 2], 16)
                    elif bk == nbk - 1:
                        # odd final bank: write only its half
                        a.wait_ge(sem["s_act"], bk + 1)
                        a.dma_start(out_d[pi][:, 0:cols], osb[pi % 2][:, 0:cols]
                                    ).then_inc(sem["s_out"][pi % 2], 16)
    nc.compile()
    return nc


def ref_numpy(x, edge_index, W, b):
    row = np.asarray(edge_index[0]).astype(np.int64)
    col = np.asarray(edge_index[1]).astype(np.int64)
    x = np.asarray(x, np.float32)
    n = x.shape[0]
    deg = np.bincount(row, minlength=n).astype(np.float32)
    dis = np.where(deg > 0, deg ** -0.5, 0.0).astype(np.float32)
    norm = dis[row] * dis[col]
    agg = np.zeros_like(x)
    np.add.at(agg, row, x[col] * norm[:, None])
    agg += x
    return agg @ np.asarray(W, np.float32).T + np.asarray(b, np.float32)


_CACHE = {}


def last_results():
    return _CACHE.get("res")


def kernel(x, edge_index, num_nodes, W, b):
    import os
    from concourse.bass_utils import run_bass_kernel_spmd

    p = FULL
    assert int(num_nodes) == p.n_nodes
    in_maps, colmap, subcap = host_prep(x, edge_index, W, b, p)
    key = tuple(int(v) for v in subcap)
    if _CACHE.get("key") != key:
        _CACHE["nc"] = build_kernel(p, subcap)
        _CACHE["key"] = key
    trace = bool(os.environ.get("GCN_TRACE"))
    res = run_bass_kernel_spmd(_CACHE["nc"], in_maps,
                               core_ids=list(range(p.n_cores)), trace=trace)
    _CACHE["res"] = res
    return assemble(res.results, p, colmap)



# revision 51
# speedup vs baseline: 1.0773x; 1.0773x over previous
"""GCNConv on 8 Trainium2 NeuronCores. Self-contained graded kernel."""


import sys
from contextlib import ExitStack
from dataclasses import dataclass

import ml_dtypes
import numpy as np

sys.path.insert(0, "/opt/trn_rl_repo")

import concourse.bacc as bacc  # noqa: E402
import concourse.mybir as mybir  # noqa: E402

BF16 = ml_dtypes.bfloat16
FP8 = ml_dtypes.float8_e4m3


@dataclass(frozen=True)
class P:
    n_nodes: int = 100000
    d: int = 128
    n_cores: int = 8
    npc: int = 12500          # nodes per core
    bd: int = 500             # destinations per bank
    nb: int = 25              # banks per core
    win: int = 16             # max dests per window (psum column block)
    nwin: int = 32            # windows per bank; nwin*win = psum bank cols

    @property
    def cols(self):
        return self.nwin * self.win


FULL = P()


def _pack_bank(cnt, nwin, win, targets=None):
    """Assign len(cnt) dests into nwin bins (<=win dests each): worst-fit
    decreasing toward the given per-bin load targets, then repair toward a
    sorted profile of [main]*(nwin-1)+[main-128] so subcap sums to the
    per-bank floor. Returns (bin id per dest, bin loads desc)."""
    nd = len(cnt)
    main = 16 * win               # bin load giving win*8 slots/sub rows...
    light = main - 128
    order = np.argsort(-cnt, kind="stable")
    if targets is None:
        targets = [main] * (nwin - 1) + [light]
    rem_e = np.asarray(targets, np.int64).copy()
    rem_d = np.full(nwin, win, np.int64)
    sums = np.zeros(nwin, np.int64)
    assign = np.empty(nd, np.int64)
    NEG = -1 << 40
    for i in order:
        c = int(cnt[i])
        feas = rem_d > 0
        b = int(np.argmax(np.where(feas, rem_e, NEG)))
        assign[i] = b
        sums[b] += c
        rem_e[b] -= c
        rem_d[b] -= 1
    counts = np.bincount(assign, minlength=nwin)
    # repair >512 bins: plain move if a bin has count room (respecting the
    # lightest bin's 384 budget), else swap a dest for a smaller one from a
    # bin with load slack (counts unchanged)
    for _ in range(128):
        wsrc = int(np.argmax(sums))
        over = int(sums[wsrc]) - main
        if over <= 0:
            break
        mem = np.where(assign == wsrc)[0]
        cand = mem[np.argsort(cnt[mem], kind="stable")]
        dst_order = np.argsort(sums, kind="stable")
        moved = False
        for i in cand:                       # plain move
            c = int(cnt[i])
            if c < over:
                continue
            for wdst in dst_order:
                if wdst == wsrc or counts[wdst] >= win:
                    continue
                lim = light if wdst == dst_order[0] else main
                if sums[wdst] + c <= lim:
                    assign[i] = wdst
                    sums[wsrc] -= c
                    sums[wdst] += c
                    counts[wsrc] -= 1
                    counts[wdst] += 1
                    moved = True
                    break
            if moved:
                break
        if not moved:                        # swap
            for wdst in dst_order:
                if wdst == wsrc:
                    continue
                lim = light if wdst == dst_order[0] else main
                slack = lim - int(sums[wdst])
                if slack < over:
                    continue
                dmem = np.where(assign == wdst)[0]
                for i in cand:
                    ci = int(cnt[i])
                    ks = dmem[(cnt[dmem] <= ci - over)
                              & (cnt[dmem] >= ci - slack)]
                    if len(ks):
                        k = ks[np.argmax(cnt[ks])]   # minimal delta
                        ck = int(cnt[k])
                        assign[i], assign[k] = wdst, wsrc
                        sums[wsrc] += ck - ci
                        sums[wdst] += ci - ck
                        moved = True
                        break
                if moved:
                    break
        if not moved:
            break
    binorder = np.argsort(-sums, kind="stable")
    remap = np.empty(nwin, np.int64)
    remap[binorder] = np.arange(nwin)
    return remap[assign], sums[binorder]


def host_prep(x, edge_index, W, b, p: P):
    """Build per-core device inputs. Returns (in_maps, colmap, subcap)."""
    n, d = p.n_nodes, p.d
    row = np.asarray(edge_index[0]).astype(np.int64)
    col = np.asarray(edge_index[1]).astype(np.int64)
    x = np.asarray(x, np.float32)
    E = row.shape[0]
    ngb = p.n_cores * p.nb

    deg = np.bincount(row, minlength=n).astype(np.float32)
    dis = np.where(deg > 0, deg ** -0.5, 0.0).astype(np.float32)
    norm = (dis[row] * dis[col]).astype(np.float32)

    # LPT-balance dests across each core's banks so every bank carries
    # ~E/(cores*nb) edges: subcap is a global max, so variance = padding.
    import heapq
    degi = np.bincount(row, minlength=n)
    bank_of = np.empty(n, np.int64)         # global bank id per node
    pos_of = np.empty(n, np.int64)          # position within bank
    for c in range(p.n_cores):
        lo = c * p.npc
        order_c = np.argsort(-degi[lo:lo + p.npc], kind="stable")
        heap = [(0, bi) for bi in range(p.nb)]
        cnt = np.zeros(p.nb, np.int64)
        for v in order_c:
            while True:
                load, bi = heapq.heappop(heap)
                if cnt[bi] < p.bd:
                    break
            bank_of[lo + v] = c * p.nb + bi
            pos_of[lo + v] = cnt[bi]
            cnt[bi] += 1
            heapq.heappush(heap, (load + int(degi[lo + v]), bi))

    gb = bank_of[row]                       # global bank id
    dloc = pos_of[row]                      # dest within bank

    # pack each bank's dests into windows
    degb = np.bincount(gb * p.bd + dloc, minlength=ngb * p.bd).reshape(ngb, p.bd)
    wof = np.empty((ngb, p.bd), np.int64)   # window of dest
    jof = np.empty((ngb, p.bd), np.int64)   # col within window
    bank_bins = np.empty((ngb, p.nwin), np.int64)
    for g in range(ngb):
        wo, sums = _pack_bank(degb[g], p.nwin, p.win)
        wof[g] = wo
        bank_bins[g] = sums
        o = np.argsort(wo, kind="stable")
        starts = np.zeros(p.nwin, np.int64)
        cnts = np.bincount(wo, minlength=p.nwin)
        starts[1:] = np.cumsum(cnts)[:-1]
        r = np.empty(p.bd, np.int64)
        r[o] = np.arange(p.bd) - starts[wo[o]]
        jof[g] = r
    assert (jof < p.win).all()

    # data-derived per-window-index sub counts (shared across cores/banks)
    subcap = np.maximum(1, -(-bank_bins.max(axis=0) // 128)).astype(np.int64)
    spb = int(subcap.sum())
    subbase = np.zeros(p.nwin, np.int64)
    subbase[1:] = np.cumsum(subcap)[:-1]

    # per-edge window / slot
    ew = wof[gb, dloc]
    ej = jof[gb, dloc]
    cell = gb * p.nwin + ew
    order = np.argsort(cell, kind="stable")
    cell_s = cell[order]
    col_s = col[order]
    norm_s = norm[order]
    ej_s = ej[order]
    gb_s = gb[order]
    ew_s = ew[order]

    cell_counts = np.bincount(cell, minlength=ngb * p.nwin)
    assert (cell_counts.reshape(ngb, p.nwin) <= subcap[None, :] * 128).all()
    cell_starts = np.zeros(ngb * p.nwin, np.int64)
    cell_starts[1:] = np.cumsum(cell_counts)[:-1]
    rank = np.arange(E) - cell_starts[cell_s]
    slot = subbase[ew_s] * 128 + rank       # slot within bank

    # norm folded into the gathered rows (single fp8 quantization); S is 0/1.
    # G and S are fused into one per-bank fp8 blob: per partition, per sub:
    # [d gathered feats | win one-hot] so one DMA streams both.
    slots = spb * 128
    G_all = np.zeros((ngb, slots, d), FP8)
    G_all[gb_s, slot] = (x[col_s] * norm_s[:, None]).astype(FP8)
    G_all = G_all.reshape(ngb, spb, 128, d).transpose(0, 2, 1, 3)

    S = np.zeros((ngb, 128, spb, p.win), FP8)
    sub = subbase[ew_s] + rank // 128
    pslot = rank % 128
    S[gb_s, pslot, sub, ej_s] = 1.0
    blob = np.concatenate([G_all, S], axis=3)  # [ngb, 128, spb, d+win]
    blob = blob.reshape(ngb, 128, spb * (d + p.win))

    # column map: (gb, 32*w + j) -> dest local id within core, else -1
    node_of = np.empty((ngb, p.bd), np.int64)
    allv = np.arange(n)
    node_of[bank_of, pos_of] = allv % p.npc
    colmap = np.full((ngb, p.cols), -1, np.int64)
    gidx = np.repeat(np.arange(ngb), p.bd)
    colmap[gidx, (wof * p.win + jof).ravel()] = node_of.ravel()
    colmap = colmap.reshape(p.n_cores, p.nb, p.cols)

    Wt = np.ascontiguousarray(np.asarray(W, np.float32).T).astype(BF16)
    bias = np.asarray(b, np.float32).reshape(d, 1)

    in_maps = []
    for c in range(p.n_cores):
        xc = x[c * p.npc:(c + 1) * p.npc]           # [npc, d]
        cm = colmap[c].reshape(-1)                   # [nb*cols]
        xTp = np.zeros((p.nb * p.cols, d), np.float32)
        used = cm >= 0
        xTp[used] = xc[cm[used]]
        xTp = np.ascontiguousarray(
            xTp.reshape(p.nb, p.cols, d).transpose(0, 2, 1)).astype(BF16)
        # self-connection rows ride the fp8 blob as raw bytes (bitcast on dev)
        xT8 = xTp.view(np.uint8).view(FP8).reshape(p.nb, 128, 2 * p.cols)
        Bc = np.concatenate([blob[c * p.nb:(c + 1) * p.nb], xT8], axis=2)
        in_maps.append({
            "B": np.ascontiguousarray(
                Bc.transpose(1, 0, 2).reshape(128, -1)),
            "Wt": Wt,
            "bias": bias,
        })
    return in_maps, colmap, subcap


def assemble(results, p: P, colmap):
    out = np.empty((p.n_cores * p.npc, p.d), np.float32)
    for c in range(p.n_cores):
        o = np.asarray(results[c]["outT"])          # [npair, d, 2*cols]
        o = o.reshape(-1, p.d, 2, p.cols).transpose(0, 2, 1, 3)
        o = o.reshape(-1, p.d, p.cols)[:p.nb]       # [nb, d, cols]
        o = o.transpose(0, 2, 1).reshape(-1, p.d)   # [nb*cols, d]
        cm = colmap[c].reshape(-1)
        used = cm >= 0
        out[c * p.npc + cm[used]] = o[used]
    return out


def build_kernel(p: P, subcap):
    nc = bacc.Bacc("TRN2", debug=False)
    dt = mybir.dt
    nbk, win, d, cols = p.nb, p.win, p.d, p.cols
    subcap = [int(v) for v in subcap]
    spb = sum(subcap)
    subbase = [0] * p.nwin
    for w in range(1, p.nwin):
        subbase[w] = subbase[w - 1] + subcap[w - 1]
    window_of_sub = []
    for w in range(p.nwin):
        window_of_sub += [w] * subcap[w]

    dw = d + win
    bpb = spb * dw + 2 * cols        # blob elems per partition per bank
    # variable chunk schedule: small chunks at start (fast ramp) + end (short
    # tail), 2-bank chunks in the middle; 5 buffers deep
    chunks = [1, 1] + [2] * ((nbk - 5) // 2) + [1, 1, 1]
    assert sum(chunks) == nbk
    nch = len(chunks)
    cstart = [0]
    for csz in chunks:
        cstart.append(cstart[-1] + csz)
    chunk_of = []
    for ci, csz in enumerate(chunks):
        chunk_of += [ci] * csz
    CMAX = max(chunks)

    B_d = nc.dram_tensor("B", [128, nbk * bpb], dt.float8e4, kind="ExternalInput")
    Wt_d = nc.dram_tensor("Wt", [d, d], dt.bfloat16, kind="ExternalInput")
    b_d = nc.dram_tensor("bias", [d, 1], dt.float32, kind="ExternalInput")
    npair = (nbk + 1) // 2
    out_d = nc.dram_tensor("outT", [npair, d, 2 * cols], dt.bfloat16,
                           kind="ExternalOutput")

    with ExitStack() as ctx:
        def sb(name, shape, dtype):
            return ctx.enter_context(nc.sbuf_tensor(name, shape, dtype))

        NB = 7                       # input-side buffer depth (chunks)
        Bsb = [sb(f"Bsb{i}", [128, CMAX * bpb], dt.float8e4) for i in range(NB)]
        ax = [sb(f"ax{i}", [128, cols], dt.bfloat16) for i in range(3)]
        # output staging: 2 ping-pong buffers x 2 banks each
        osb = [sb(f"osb{i}", [128, 2 * cols], dt.bfloat16) for i in range(2)]
        Wt_sb = sb("Wt_sb", [128, d], dt.bfloat16)
        b_sb = sb("b_sb", [128, 1], dt.float32)
        pagg = [ctx.enter_context(nc.psum_tensor(f"pagg{i}", [128, cols], dt.float32))
                for i in range(3)]
        pfin = [ctx.enter_context(nc.psum_tensor(f"pfin{i}", [128, cols], dt.float32))
                for i in range(3)]

        names = ["s_const", "s_peb", "s_dve", "s_fin", "s_act"]
        sem = {nm: ctx.enter_context(nc.semaphore(nm)) for nm in names}
        # parity-split DMA sems: one sem per buffer slot, so at most one
        # in-flight DMA per sem and wait values are unambiguous
        sem["s_b"] = [ctx.enter_context(nc.semaphore(f"s_b{i}")) for i in range(NB)]
        sem["s_b0a"] = ctx.enter_context(nc.semaphore("s_b0a"))
        sem["s_bza"] = ctx.enter_context(nc.semaphore("s_bza"))
        sem["s_out"] = [ctx.enter_context(nc.semaphore(f"s_out{i}")) for i in range(2)]
        half0 = (spb // 2) * dw      # bank-0 first-half split point (elems)

        with nc.Block() as block:
            @block.sync
            def _(s):
                for j in range(nch):
                    if j >= NB:
                        # buffer j%NB free once DVE consumed chunk j-NB fully
                        s.wait_ge(sem["s_dve"], cstart[j - NB] + chunks[j - NB])
                    lo, hi = cstart[j], cstart[j + 1]
                    if j == 0:
                        # split bank 0 so the PE can start on the first half
                        s.dma_start(Bsb[0][:, 0:half0], B_d[:, 0:half0]
                                    ).then_inc(sem["s_b0a"], 16)
                        s.dma_start(Bsb[0][:, half0:bpb], B_d[:, half0:bpb]
                                    ).then_inc(sem["s_b"][0], 16)
                        continue
                    if j == nch - 1:
                        # split the last bank too: PE starts on its first half
                        # while the second half streams
                        zb = lo * bpb
                        s.dma_start(Bsb[j % NB][:, 0:half0],
                                    B_d[:, zb:zb + half0]
                                    ).then_inc(sem["s_bza"], 16)
                        s.dma_start(Bsb[j % NB][:, half0:bpb],
                                    B_d[:, zb + half0:zb + bpb]
                                    ).then_inc(sem["s_b"][j % NB], 16)
                        continue
                    s.dma_start(Bsb[j % NB][:, 0:(hi - lo) * bpb],
                                B_d[:, lo * bpb:hi * bpb]
                                ).then_inc(sem["s_b"][j % NB], 16)

            @block.tensor
            def _(pe):
                def final_mm(fb):
                    pe.wait_ge(sem["s_dve"], fb + 1)
                    if fb == 0:
                        pe.wait_ge(sem["s_const"], 32)
                    if fb >= 3:
                        pe.wait_ge(sem["s_act"], fb - 2)
                    nc.tensor.matmul(
                        pfin[fb % 3][:, :], Wt_sb[:, :], ax[fb % 3][:, :],
                        start=True, stop=True,
                    ).then_inc(sem["s_fin"], 1)

                for bk in range(nbk):
                    cj = chunk_of[bk]
                    lane = bk - cstart[cj]
                    if bk == 0:
                        pe.wait_ge(sem["s_b0a"], 16)
                    elif bk == nbk - 1:
                        pe.wait_ge(sem["s_bza"], 16)
                    elif lane == 0:
                        pe.wait_ge(sem["s_b"][cj % NB], 16 * (cj // NB + 1))
                    if bk >= 3:
                        pe.wait_ge(sem["s_dve"], bk - 2)
                    base = lane * bpb
                    mm = None
                    for si in range(spb):
                        if (bk == 0 or bk == nbk - 1) and si == spb // 2:
                            pe.wait_ge(sem["s_b"][(0 if bk == 0 else
                                                   (nch - 1) % NB)],
                                       16 * (1 if bk == 0 else
                                             ((nch - 1) // NB + 1)))
                        w = window_of_sub[si]
                        jj = si - subbase[w]
                        mm = nc.tensor.matmul(
                            pagg[bk % 3][:, w * win:(w + 1) * win],
                            Bsb[cj % NB][:, base + si * dw:base + si * dw + d],
                            Bsb[cj % NB][:, base + si * dw + d:base + (si + 1) * dw],
                            start=(jj == 0), stop=(jj == subcap[w] - 1),
                        )
                    mm.then_inc(sem["s_peb"], 1)
                    if bk >= 1:
                        final_mm(bk - 1)
                final_mm(nbk - 1)

            @block.vector
            def _(v):
                for bk in range(nbk):
                    cj = chunk_of[bk]
                    base = (bk - cstart[cj]) * bpb + spb * dw
                    v.wait_ge(sem["s_peb"], bk + 1)
                    if bk >= 3:
                        v.wait_ge(sem["s_fin"], bk - 2)
                    xv = Bsb[cj % NB][:, base:base + 2 * cols].bitcast(dt.bfloat16)
                    nc.vector.tensor_add(
                        ax[bk % 3][:, :], pagg[bk % 3][:, :], xv
                    ).then_inc(sem["s_dve"], 1)

            @block.scalar
            def _(a):
                a.dma_start(Wt_sb[:, :], Wt_d[:, :]).then_inc(sem["s_const"], 16)
                a.dma_start(b_sb[:, :], b_d[:, :]).then_inc(sem["s_const"], 16)
                a.wait_ge(sem["s_const"], 32)
                for bk in range(nbk):
                    pi, lane = bk // 2, bk % 2
                    a.wait_ge(sem["s_fin"], bk + 1)
                    if lane == 0 and pi >= 2:
                        a.wait_ge(sem["s_out"][pi % 2], 16 * (pi // 2))
                    nc.scalar.activation(
                        osb[pi % 2][:, lane * cols:(lane + 1) * cols],
                        pfin[bk % 3][:, :],
                        mybir.ActivationFunctionType.Identity, bias=b_sb[:, :],
                    ).then_inc(sem["s_act"], 1)
                    if lane == 1:
                        a.wait_ge(sem["s_act"], bk + 1)
                        a.dma_start(out_d[pi], osb[pi % 2][:, :]
                                    ).then_inc(sem["s_out"][pi % 2], 16)
                    elif bk == nbk - 1:
                        # odd final bank: write only its half
                        a.wait_ge(sem["s_act"], bk + 1)
                        a.dma_start(out_d[pi][:, 0:cols], osb[pi % 2][:, 0:cols]
                                    ).then_inc(sem["s_out"][pi % 2], 16)
    nc.compile()
    return nc


def ref_numpy(x, edge_index, W, b):
    row = np.asarray(edge_index[0]).astype(np.int64)
    col = np.asarray(edge_index[1]).astype(np.int64)
    x = np.asarray(x, np.float32)
    n = x.shape[0]
    deg = np.bincount(row, minlength=n).astype(np.float32)
    dis = np.where(deg > 0, deg ** -0.5, 0.0).astype(np.float32)
    norm = dis[row] * dis[col]
    agg = np.zeros_like(x)
    np.add.at(agg, row, x[col] * norm[:, None])
    agg += x
    return agg @ np.asarray(W, np.float32).T + np.asarray(b, np.float32)


_CACHE = {}


def last_results():
    return _CACHE.get("res")


def kernel(x, edge_index, num_nodes, W, b):
    import os
    from concourse.bass_utils import run_bass_kernel_spmd

    p = FULL
    assert int(num_nodes) == p.n_nodes
    in_maps, colmap, subcap = host_prep(x, edge_index, W, b, p)
    key = tuple(int(v) for v in subcap)
    if _CACHE.get("key") != key:
        _CACHE["nc"] = build_kernel(p, subcap)
        _CACHE["key"] = key
    trace = bool(os.environ.get("GCN_TRACE"))
    res = run_bass_kernel_spmd(_CACHE["nc"], in_maps,
                               core_ids=list(range(p.n_cores)), trace=trace)
    _CACHE["res"] = res
    return assemble(res.results, p, colmap)



# revision 56
# speedup vs baseline: 1.0885x; 1.0103x over previous
"""GCNConv on 8 Trainium2 NeuronCores. Self-contained graded kernel."""


import sys
from contextlib import ExitStack
from dataclasses import dataclass

import ml_dtypes
import numpy as np

sys.path.insert(0, "/opt/trn_rl_repo")

import concourse.bacc as bacc  # noqa: E402
import concourse.mybir as mybir  # noqa: E402

BF16 = ml_dtypes.bfloat16
FP8 = ml_dtypes.float8_e4m3


@dataclass(frozen=True)
class P:
    n_nodes: int = 100000
    d: int = 128
    n_cores: int = 8
    npc: int = 12500          # nodes per core
    bd: int = 500             # destinations per bank
    nb: int = 25              # banks per core
    win: int = 16             # max dests per window (psum column block)
    nwin: int = 32            # windows per bank; nwin*win = psum bank cols

    @property
    def cols(self):
        return self.nwin * self.win


FULL = P()


def _chunks(nbk):
    """DMA chunk schedule in banks: fast ramp, 2-bank middle, short tail."""
    return [1, 1] + [2] * ((nbk - 5) // 2) + [1, 1, 1]


def _pack_bank(cnt, nwin, win, targets=None):
    """Assign len(cnt) dests into nwin bins (<=win dests each): worst-fit
    decreasing toward the given per-bin load targets, then repair toward a
    sorted profile of [main]*(nwin-1)+[main-128] so subcap sums to the
    per-bank floor. Returns (bin id per dest, bin loads desc)."""
    nd = len(cnt)
    main = 16 * win               # bin load giving win*8 slots/sub rows...
    light = main - 128
    order = np.argsort(-cnt, kind="stable")
    if targets is None:
        targets = [main] * (nwin - 1) + [light]
    rem_e = np.asarray(targets, np.int64).copy()
    rem_d = np.full(nwin, win, np.int64)
    sums = np.zeros(nwin, np.int64)
    assign = np.empty(nd, np.int64)
    NEG = -1 << 40
    for i in order:
        c = int(cnt[i])
        feas = rem_d > 0
        b = int(np.argmax(np.where(feas, rem_e, NEG)))
        assign[i] = b
        sums[b] += c
        rem_e[b] -= c
        rem_d[b] -= 1
    counts = np.bincount(assign, minlength=nwin)
    # repair >512 bins: plain move if a bin has count room (respecting the
    # lightest bin's 384 budget), else swap a dest for a smaller one from a
    # bin with load slack (counts unchanged)
    for _ in range(128):
        wsrc = int(np.argmax(sums))
        over = int(sums[wsrc]) - main
        if over <= 0:
            break
        mem = np.where(assign == wsrc)[0]
        cand = mem[np.argsort(cnt[mem], kind="stable")]
        dst_order = np.argsort(sums, kind="stable")
        moved = False
        for i in cand:                       # plain move
            c = int(cnt[i])
            if c < over:
                continue
            for wdst in dst_order:
                if wdst == wsrc or counts[wdst] >= win:
                    continue
                lim = light if wdst == dst_order[0] else main
                if sums[wdst] + c <= lim:
                    assign[i] = wdst
                    sums[wsrc] -= c
                    sums[wdst] += c
                    counts[wsrc] -= 1
                    counts[wdst] += 1
                    moved = True
                    break
            if moved:
                break
        if not moved:                        # swap
            for wdst in dst_order:
                if wdst == wsrc:
                    continue
                lim = light if wdst == dst_order[0] else main
                slack = lim - int(sums[wdst])
                if slack < over:
                    continue
                dmem = np.where(assign == wdst)[0]
                for i in cand:
                    ci = int(cnt[i])
                    ks = dmem[(cnt[dmem] <= ci - over)
                              & (cnt[dmem] >= ci - slack)]
                    if len(ks):
                        k = ks[np.argmax(cnt[ks])]   # minimal delta
                        ck = int(cnt[k])
                        assign[i], assign[k] = wdst, wsrc
                        sums[wsrc] += ck - ci
                        sums[wdst] += ci - ck
                        moved = True
                        break
                if moved:
                    break
        if not moved:
            break
    binorder = np.argsort(-sums, kind="stable")
    remap = np.empty(nwin, np.int64)
    remap[binorder] = np.arange(nwin)
    return remap[assign], sums[binorder]


def host_prep(x, edge_index, W, b, p: P):
    """Build per-core device inputs. Returns (in_maps, colmap, subcap)."""
    n, d = p.n_nodes, p.d
    row = np.asarray(edge_index[0]).astype(np.int64)
    col = np.asarray(edge_index[1]).astype(np.int64)
    x = np.asarray(x, np.float32)
    E = row.shape[0]
    ngb = p.n_cores * p.nb

    deg = np.bincount(row, minlength=n).astype(np.float32)
    dis = np.where(deg > 0, deg ** -0.5, 0.0).astype(np.float32)
    norm = (dis[row] * dis[col]).astype(np.float32)

    # LPT-balance dests across each core's banks so every bank carries
    # ~E/(cores*nb) edges: subcap is a global max, so variance = padding.
    import heapq
    degi = np.bincount(row, minlength=n)
    bank_of = np.empty(n, np.int64)         # global bank id per node
    pos_of = np.empty(n, np.int64)          # position within bank
    for c in range(p.n_cores):
        lo = c * p.npc
        order_c = np.argsort(-degi[lo:lo + p.npc], kind="stable")
        heap = [(0, bi) for bi in range(p.nb)]
        cnt = np.zeros(p.nb, np.int64)
        for v in order_c:
            while True:
                load, bi = heapq.heappop(heap)
                if cnt[bi] < p.bd:
                    break
            bank_of[lo + v] = c * p.nb + bi
            pos_of[lo + v] = cnt[bi]
            cnt[bi] += 1
            heapq.heappush(heap, (load + int(degi[lo + v]), bi))

    gb = bank_of[row]                       # global bank id
    dloc = pos_of[row]                      # dest within bank

    # pack each bank's dests into windows
    degb = np.bincount(gb * p.bd + dloc, minlength=ngb * p.bd).reshape(ngb, p.bd)
    wof = np.empty((ngb, p.bd), np.int64)   # window of dest
    jof = np.empty((ngb, p.bd), np.int64)   # col within window
    bank_bins = np.empty((ngb, p.nwin), np.int64)
    for g in range(ngb):
        wo, sums = _pack_bank(degb[g], p.nwin, p.win)
        wof[g] = wo
        bank_bins[g] = sums
        o = np.argsort(wo, kind="stable")
        starts = np.zeros(p.nwin, np.int64)
        cnts = np.bincount(wo, minlength=p.nwin)
        starts[1:] = np.cumsum(cnts)[:-1]
        r = np.empty(p.bd, np.int64)
        r[o] = np.arange(p.bd) - starts[wo[o]]
        jof[g] = r
    assert (jof < p.win).all()

    # data-derived per-window-index sub counts (shared across cores/banks)
    subcap = np.maximum(1, -(-bank_bins.max(axis=0) // 128)).astype(np.int64)
    spb = int(subcap.sum())
    subbase = np.zeros(p.nwin, np.int64)
    subbase[1:] = np.cumsum(subcap)[:-1]

    # per-edge window / slot
    ew = wof[gb, dloc]
    ej = jof[gb, dloc]
    cell = gb * p.nwin + ew
    order = np.argsort(cell, kind="stable")
    cell_s = cell[order]
    col_s = col[order]
    norm_s = norm[order]
    ej_s = ej[order]
    gb_s = gb[order]
    ew_s = ew[order]

    cell_counts = np.bincount(cell, minlength=ngb * p.nwin)
    assert (cell_counts.reshape(ngb, p.nwin) <= subcap[None, :] * 128).all()
    cell_starts = np.zeros(ngb * p.nwin, np.int64)
    cell_starts[1:] = np.cumsum(cell_counts)[:-1]
    rank = np.arange(E) - cell_starts[cell_s]
    slot = subbase[ew_s] * 128 + rank       # slot within bank

    # norm folded into the gathered rows (single fp8 quantization); S is 0/1.
    # G and S are fused into one per-bank fp8 blob: per partition, per sub:
    # [d gathered feats | win one-hot] so one DMA streams both.
    slots = spb * 128
    G_all = np.zeros((ngb, slots, d), FP8)
    G_all[gb_s, slot] = (x[col_s] * norm_s[:, None]).astype(FP8)
    G_all = G_all.reshape(ngb, spb, 128, d).transpose(0, 2, 1, 3)

    S = np.zeros((ngb, 128, spb, p.win), FP8)
    sub = subbase[ew_s] + rank // 128
    pslot = rank % 128
    S[gb_s, pslot, sub, ej_s] = 1.0
    blob = np.concatenate([G_all, S], axis=3)  # [ngb, 128, spb, d+win]
    blob = blob.reshape(ngb, 128, spb * (d + p.win))

    # column map: (gb, 32*w + j) -> dest local id within core, else -1
    node_of = np.empty((ngb, p.bd), np.int64)
    allv = np.arange(n)
    node_of[bank_of, pos_of] = allv % p.npc
    colmap = np.full((ngb, p.cols), -1, np.int64)
    gidx = np.repeat(np.arange(ngb), p.bd)
    colmap[gidx, (wof * p.win + jof).ravel()] = node_of.ravel()
    colmap = colmap.reshape(p.n_cores, p.nb, p.cols)

    Wt = np.ascontiguousarray(np.asarray(W, np.float32).T).astype(BF16)
    bias = np.asarray(b, np.float32).reshape(d, 1)

    in_maps = []
    for c in range(p.n_cores):
        xc = x[c * p.npc:(c + 1) * p.npc]           # [npc, d]
        cm = colmap[c].reshape(-1)                   # [nb*cols]
        xTp = np.zeros((p.nb * p.cols, d), np.float32)
        used = cm >= 0
        xTp[used] = xc[cm[used]]
        xTp = np.ascontiguousarray(
            xTp.reshape(p.nb, p.cols, d).transpose(0, 2, 1)).astype(BF16)
        # self-connection rows ride the fp8 blob as raw bytes (bitcast on dev)
        xT8 = xTp.view(np.uint8).view(FP8).reshape(p.nb, 128, 2 * p.cols)
        Bc = np.concatenate([blob[c * p.nb:(c + 1) * p.nb], xT8], axis=2)
        Bf = Bc.transpose(1, 0, 2).reshape(128, -1)  # [128, nb*bpb]
        # chunk-major HBM layout: each chunk's partition runs are adjacent
        bpb = Bc.shape[2]
        ones, twos, lo = [], [], 0
        for csz in _chunks(p.nb):
            blk = Bf[:, lo * bpb:(lo + csz) * bpb]
            (ones if csz == 1 else twos).append(blk)
            lo += csz
        in_maps.append({
            "B1": np.ascontiguousarray(np.stack(ones)),
            "B2": np.ascontiguousarray(np.stack(twos)),
            "Wt": Wt,
            "bias": bias,
        })
    return in_maps, colmap, subcap


def assemble(results, p: P, colmap):
    out = np.empty((p.n_cores * p.npc, p.d), np.float32)
    for c in range(p.n_cores):
        o = np.asarray(results[c]["outT"])          # [npair, d, 2*cols]
        o = o.reshape(-1, p.d, 2, p.cols).transpose(0, 2, 1, 3)
        o = o.reshape(-1, p.d, p.cols)[:p.nb]       # [nb, d, cols]
        o = o.transpose(0, 2, 1).reshape(-1, p.d)   # [nb*cols, d]
        cm = colmap[c].reshape(-1)
        used = cm >= 0
        out[c * p.npc + cm[used]] = o[used]
    return out


def build_kernel(p: P, subcap):
    nc = bacc.Bacc("TRN2", debug=False)
    dt = mybir.dt
    nbk, win, d, cols = p.nb, p.win, p.d, p.cols
    subcap = [int(v) for v in subcap]
    spb = sum(subcap)
    subbase = [0] * p.nwin
    for w in range(1, p.nwin):
        subbase[w] = subbase[w - 1] + subcap[w - 1]
    window_of_sub = []
    for w in range(p.nwin):
        window_of_sub += [w] * subcap[w]

    dw = d + win
    bpb = spb * dw + 2 * cols        # blob elems per partition per bank
    chunks = _chunks(nbk)
    assert sum(chunks) == nbk
    nch = len(chunks)
    n1 = sum(1 for c in chunks if c == 1)
    n2 = nch - n1
    idx_of = []                      # chunk j -> index within its size class
    c1 = c2 = 0
    for csz in chunks:
        if csz == 1:
            idx_of.append(c1)
            c1 += 1
        else:
            idx_of.append(c2)
            c2 += 1
    cstart = [0]
    for csz in chunks:
        cstart.append(cstart[-1] + csz)
    chunk_of = []
    for ci, csz in enumerate(chunks):
        chunk_of += [ci] * csz
    CMAX = max(chunks)

    B1_d = nc.dram_tensor("B1", [n1, 128, bpb], dt.float8e4, kind="ExternalInput")
    B2_d = nc.dram_tensor("B2", [n2, 128, 2 * bpb], dt.float8e4,
                          kind="ExternalInput")
    Wt_d = nc.dram_tensor("Wt", [d, d], dt.bfloat16, kind="ExternalInput")
    b_d = nc.dram_tensor("bias", [d, 1], dt.float32, kind="ExternalInput")
    npair = (nbk + 1) // 2
    out_d = nc.dram_tensor("outT", [npair, d, 2 * cols], dt.bfloat16,
                           kind="ExternalOutput")

    with ExitStack() as ctx:
        def sb(name, shape, dtype):
            return ctx.enter_context(nc.sbuf_tensor(name, shape, dtype))

        NB = 7                       # input-side buffer depth (chunks)
        Bsb = [sb(f"Bsb{i}", [128, CMAX * bpb], dt.float8e4) for i in range(NB)]
        ax = [sb(f"ax{i}", [128, cols], dt.bfloat16) for i in range(3)]
        # output staging: 2 ping-pong buffers x 2 banks each
        osb = [sb(f"osb{i}", [128, 2 * cols], dt.bfloat16) for i in range(2)]
        Wt_sb = sb("Wt_sb", [128, d], dt.bfloat16)
        b_sb = sb("b_sb", [128, 1], dt.float32)
        pagg = [ctx.enter_context(nc.psum_tensor(f"pagg{i}", [128, cols], dt.float32))
                for i in range(3)]
        pfin = [ctx.enter_context(nc.psum_tensor(f"pfin{i}", [128, cols], dt.float32))
                for i in range(3)]

        names = ["s_const", "s_peb", "s_dve", "s_fin", "s_act"]
        sem = {nm: ctx.enter_context(nc.semaphore(nm)) for nm in names}
        # parity-split DMA sems: one sem per buffer slot, so at most one
        # in-flight DMA per sem and wait values are unambiguous
        sem["s_b"] = [ctx.enter_context(nc.semaphore(f"s_b{i}")) for i in range(NB)]
        sem["s_b0a"] = ctx.enter_context(nc.semaphore("s_b0a"))
        sem["s_bza"] = ctx.enter_context(nc.semaphore("s_bza"))
        sem["s_out"] = [ctx.enter_context(nc.semaphore(f"s_out{i}")) for i in range(2)]
        half0 = (spb // 2) * dw      # bank-0 first-half split point (elems)

        with nc.Block() as block:
            @block.sync
            def _(s):
                for j in range(nch):
                    if j >= NB:
                        # buffer j%NB free once DVE consumed chunk j-NB fully
                        s.wait_ge(sem["s_dve"], cstart[j - NB] + chunks[j - NB])
                    src1 = B1_d[idx_of[j]] if chunks[j] == 1 else None
                    if j == 0:
                        # split bank 0 so the PE can start on the first half
                        s.dma_start(Bsb[0][:, 0:half0], src1[:, 0:half0]
                                    ).then_inc(sem["s_b0a"], 16)
                        s.dma_start(Bsb[0][:, half0:bpb], src1[:, half0:bpb]
                                    ).then_inc(sem["s_b"][0], 16)
                        continue
                    if j == nch - 1:
                        # split the last bank too: PE starts on its first half
                        # while the second half streams
                        s.dma_start(Bsb[j % NB][:, 0:half0], src1[:, 0:half0]
                                    ).then_inc(sem["s_bza"], 16)
                        s.dma_start(Bsb[j % NB][:, half0:bpb],
                                    src1[:, half0:bpb]
                                    ).then_inc(sem["s_b"][j % NB], 16)
                        continue
                    if chunks[j] == 1:
                        s.dma_start(Bsb[j % NB][:, 0:bpb], src1
                                    ).then_inc(sem["s_b"][j % NB], 16)
                    else:
                        s.dma_start(Bsb[j % NB][:, 0:2 * bpb], B2_d[idx_of[j]]
                                    ).then_inc(sem["s_b"][j % NB], 16)

            @block.tensor
            def _(pe):
                def final_mm(fb):
                    pe.wait_ge(sem["s_dve"], fb + 1)
                    if fb == 0:
                        pe.wait_ge(sem["s_const"], 32)
                    if fb >= 3:
                        pe.wait_ge(sem["s_act"], fb - 2)
                    nc.tensor.matmul(
                        pfin[fb % 3][:, :], Wt_sb[:, :], ax[fb % 3][:, :],
                        start=True, stop=True,
                    ).then_inc(sem["s_fin"], 1)

                for bk in range(nbk):
                    cj = chunk_of[bk]
                    lane = bk - cstart[cj]
                    if bk == 0:
                        pe.wait_ge(sem["s_b0a"], 16)
                    elif bk == nbk - 1:
                        pe.wait_ge(sem["s_bza"], 16)
                    elif lane == 0:
                        pe.wait_ge(sem["s_b"][cj % NB], 16 * (cj // NB + 1))
                    if bk >= 3:
                        pe.wait_ge(sem["s_dve"], bk - 2)
                    base = lane * bpb
                    mm = None
                    for si in range(spb):
                        if (bk == 0 or bk == nbk - 1) and si == spb // 2:
                            pe.wait_ge(sem["s_b"][(0 if bk == 0 else
                                                   (nch - 1) % NB)],
                                       16 * (1 if bk == 0 else
                                             ((nch - 1) // NB + 1)))
                        w = window_of_sub[si]
                        jj = si - subbase[w]
                        mm = nc.tensor.matmul(
                            pagg[bk % 3][:, w * win:(w + 1) * win],
                            Bsb[cj % NB][:, base + si * dw:base + si * dw + d],
                            Bsb[cj % NB][:, base + si * dw + d:base + (si + 1) * dw],
                            start=(jj == 0), stop=(jj == subcap[w] - 1),
                        )
                    mm.then_inc(sem["s_peb"], 1)
                    if bk >= 1:
                        final_mm(bk - 1)
                final_mm(nbk - 1)

            @block.vector
            def _(v):
                for bk in range(nbk):
                    cj = chunk_of[bk]
                    base = (bk - cstart[cj]) * bpb + spb * dw
                    v.wait_ge(sem["s_peb"], bk + 1)
                    if bk >= 3:
                        v.wait_ge(sem["s_fin"], bk - 2)
                    xv = Bsb[cj % NB][:, base:base + 2 * cols].bitcast(dt.bfloat16)
                    nc.vector.tensor_add(
                        ax[bk % 3][:, :], pagg[bk % 3][:, :], xv
                    ).then_inc(sem["s_dve"], 1)

            @block.scalar
            def _(a):
                a.dma_start(Wt_sb[:, :], Wt_d[:, :]).then_inc(sem["s_const"], 16)
                a.dma_start(b_sb[:, :], b_d[:, :]).then_inc(sem["s_const"], 16)
                a.wait_ge(sem["s_const"], 32)
                for bk in range(nbk):
                    pi, lane = bk // 2, bk % 2
                    a.wait_ge(sem["s_fin"], bk + 1)
                    if lane == 0 and pi >= 2:
                        a.wait_ge(sem["s_out"][pi % 2], 16 * (pi // 2))
                    nc.scalar.activation(
                        osb[pi % 2][:, lane * cols:(lane + 1) * cols],
                        pfin[bk % 3][:, :],
                        mybir.ActivationFunctionType.Identity, bias=b_sb[:, :],
                    ).then_inc(sem["s_act"], 1)
                    if lane == 1:
                        a.wait_ge(sem["s_act"], bk + 1)
                        a.dma_start(out_d[pi], osb[pi % 2][:, :]
                                    ).then_inc(sem["s_out"][pi % 2], 16)
                    elif bk == nbk - 1:
                        # odd final bank: write only its half
                        a.wait_ge(sem["s_act"], bk + 1)
                        a.dma_start(out_d[pi][:, 0:cols], osb[pi % 2][:, 0:cols]
                                    ).then_inc(sem["s_out"][pi % 2], 16)
    nc.compile()
    return nc


def ref_numpy(x, edge_index, W, b):
    row = np.asarray(edge_index[0]).astype(np.int64)
    col = np.asarray(edge_index[1]).astype(np.int64)
    x = np.asarray(x, np.float32)
    n = x.shape[0]
    deg = np.bincount(row, minlength=n).astype(np.float32)
    dis = np.where(deg > 0, deg ** -0.5, 0.0).astype(np.float32)
    norm = dis[row] * dis[col]
    agg = np.zeros_like(x)
    np.add.at(agg, row, x[col] * norm[:, None])
    agg += x
    return agg @ np.asarray(W, np.float32).T + np.asarray(b, np.float32)


_CACHE = {}


def last_results():
    return _CACHE.get("res")


def kernel(x, edge_index, num_nodes, W, b):
    import os
    from concourse.bass_utils import run_bass_kernel_spmd

    p = FULL
    assert int(num_nodes) == p.n_nodes
    in_maps, colmap, subcap = host_prep(x, edge_index, W, b, p)
    key = tuple(int(v) for v in subcap)
    if _CACHE.get("key") != key:
        _CACHE["nc"] = build_kernel(p, subcap)
        _CACHE["key"] = key
    trace = bool(os.environ.get("GCN_TRACE"))
    res = run_bass_kernel_spmd(_CACHE["nc"], in_maps,
                               core_ids=list(range(p.n_cores)), trace=trace)
    _CACHE["res"] = res
    return assemble(res.results, p, colmap)

